# revision 1
# baseline (speedup 1.0000x reference)
"""Distributed causal attention w/ RoPE for TRN2 (8 NeuronCores).

Sharding: tensor-parallel over heads (2 heads/core). Per core:
  - QKV projection of the full sequence for its 2 heads, computed in
    transposed layout (qkv^T = W^T.T @ x^T) so attention matmuls need no
    on-device transposes of q/k.
  - RoPE applied via pair-swapped duplicate projections (host permutes
    weight rows) + elementwise DVE ops.
  - Causal attention per (batch, head, 512-token query group), scores
    computed transposed [tk, tq] so the AV matmul consumes natural-layout
    V tiles; softmax denominators via an appended ones-column in V.
  - Output projection partials reduced across cores with 8 chunked
    ReduceScatters (one per (batch, query-group)) overlapped with compute.
Host side: input layout prep (transposes/permutes of x and weights) and
concatenation of the disjoint ReduceScatter shards.
"""

import numpy as np

import concourse.bass as bass
import concourse.bacc as bacc
import concourse.mybir as mybir
from concourse import tile
from concourse.bass_utils import run_bass_kernel_spmd

B, T, C, H, D = 2, 2048, 1024, 16, 64
NCORE = 8
HPC = H // NCORE          # heads per core = 2
TCH = 512                 # token chunk (qkv proj free dim & query group)
NTC = T // TCH            # 4
NBLK = T // 128           # 16 tk tiles per batch
ROPE_BASE = 10000.0
F32 = mybir.dt.float32
F16 = mybir.dt.float16


def _rope_tables():
    # row p of an m-tile holds head_local = p // 64, d = p % 64
    d = np.arange(D)
    j = d // 2
    theta = ROPE_BASE ** (-(2.0 * j) / D)          # per-row theta
    t = np.arange(T, dtype=np.float64)
    ang = t[None, :] * theta[:, None]              # [64, T]
    cos = np.cos(ang)
    sin = np.sin(ang)
    sgn = np.where(d % 2 == 0, -1.0, 1.0)[:, None]
    c1 = np.concatenate([cos, cos], axis=0)        # [128, T]
    s1 = np.concatenate([sgn * sin, sgn * sin], axis=0)
    scale = 1.0 / np.sqrt(D)
    return (
        (c1 * scale).astype(np.float16),
        (s1 * scale).astype(np.float16),
        c1.astype(np.float16),
        s1.astype(np.float16),
    )


def _masks():
    # mask[r][tk_local, tq_local] = 1 if tq_local >= 128*r + tk_local
    out = []
    for r in range(4):
        tk = np.arange(128)[:, None]
        tq = np.arange(TCH)[None, :]
        out.append((tq >= 128 * r + tk).astype(np.float32))
    return out


def build(debug=False):
    nc = bacc.Bacc(num_devices=NCORE)
    x_t = nc.declare_dram_parameter("x_t", [B, C, T], F16, isOutput=False)
    w_all = nc.declare_dram_parameter("w_all", [C, 384], F16, isOutput=False)
    w_p = nc.declare_dram_parameter("w_p", [128, C], F16, isOutput=False)
    out_ext = nc.declare_dram_parameter("out", [B * NTC, 128, TCH], F16,
                                        isOutput=True)
    dbg = {}
    if debug:
        dbg["rope_q"] = nc.declare_dram_parameter("dbg_rope_q", [128, T], F16,
                                                  isOutput=True)
        dbg["rope_k"] = nc.declare_dram_parameter("dbg_rope_k", [128, T], F16,
                                                  isOutput=True)
        dbg["vaug"] = nc.declare_dram_parameter("dbg_vaug", [128, NBLK * 64],
                                                F16, isOutput=True)
        dbg["exp"] = nc.declare_dram_parameter("dbg_exp", [128, TCH], F16,
                                               isOutput=True)
        dbg["den"] = nc.declare_dram_parameter("dbg_den", [64, TCH], F32,
                                               isOutput=True)
        dbg["rbc"] = nc.declare_dram_parameter("dbg_rbc", [64, TCH], F32,
                                               isOutput=True)
        dbg["yh"] = nc.declare_dram_parameter("dbg_yh", [64, TCH], F16,
                                              isOutput=True)

    cq_np, sq_np, ck_np, sk_np = _rope_tables()
    mask_np = _masks()
    ident_np = np.concatenate([np.eye(64, dtype=np.float16)] * 2, axis=0)
    ident_c = nc.inline_tensor(ident_np, name="ident")  # [128, 64]
    cq_c = nc.inline_tensor(cq_np, name="cq")
    sq_c = nc.inline_tensor(sq_np, name="sq")
    ck_c = nc.inline_tensor(ck_np, name="ck")
    sk_c = nc.inline_tensor(sk_np, name="sk")
    mask_c = [nc.inline_tensor(mask_np[r].astype(np.float16), name=f"mask{r}") for r in range(4)]
    ones_c = nc.inline_tensor(np.ones((128, 64), np.float16), name="ones")

    cc_in = [nc.dram_tensor(f"cc_in{j}", [C, TCH], F16) for j in range(B * NTC)]
    cc_out = [nc.dram_tensor(f"cc_out{j}", [128, TCH], F16)
              for j in range(B * NTC)]
    cc_half = [nc.dram_tensor(f"cc_half{j}", [128, TCH // 2], F16)
               for j in range(2)]
    cc_in_half = [nc.dram_tensor(f"cc_inh{j}", [C, TCH // 2], F16)
                  for j in range(2)]
    groups = [list(range(NCORE))]

    with tile.TileContext(nc) as tc:
        with (
            tc.tile_pool(name="const", bufs=1) as cpool,
            tc.tile_pool(name="big", bufs=2) as bpool,
            tc.tile_pool(name="xt", bufs=16) as xpool,
            tc.tile_pool(name="tmp", bufs=4) as tpool,
            tc.tile_pool(name="exp", bufs=6) as epool,
            tc.tile_pool(name="ysmall", bufs=4) as ypool_sb,
            tc.tile_pool(name="mm", bufs=4, space="PSUM") as mmpool,
            tc.tile_pool(name="ypsum", bufs=1, space="PSUM") as ypool,
            tc.tile_pool(name="tpsum", bufs=1, space="PSUM") as tppool,
            tc.tile_pool(name="dpsum", bufs=2, space="PSUM") as dpool,
        ):
            # ---- persistent SBUF loads (weights first: unblock matmuls) ----
            w_sb = []
            for c in range(8):
                w = cpool.tile([128, 384], F16, tag=f"w{c}")
                nc.sync.dma_start(out=w[:, :], in_=w_all[c * 128:(c + 1) * 128, :])
                w_sb.append(w)
            wp_sb = cpool.tile([128, C], F16, tag="wp")
            nc.sync.dma_start(out=wp_sb[:, :], in_=w_p[:, :])
            cq_sb = cpool.tile([128, T], F16, tag="cq")
            sq_sb = cpool.tile([128, T], F16, tag="sq")
            ck_sb = cpool.tile([128, T], F16, tag="ck")
            sk_sb = cpool.tile([128, T], F16, tag="sk")
            ident_sb = cpool.tile([128, 64], F16, tag="ident")
            nc.sync.dma_start(out=ident_sb[:, :], in_=ident_c[:, :])
            mask_sb = []
            for r in range(4):
                m = cpool.tile([128, TCH], F16, tag=f"mask{r}")
                nc.sync.dma_start(out=m[:, :], in_=mask_c[r][:, :])
                mask_sb.append(m)
            ones_sb = cpool.tile([128, 64], F16, tag="ones")
            nc.sync.dma_start(out=ones_sb[:, :], in_=ones_c[:, :])

            for b in range(B):
                # ---- QKV projection + fused RoPE ---------------------------
                rope_q = bpool.tile([128, T], F16, tag="rope_q")
                rope_k = bpool.tile([128, T], F16, tag="rope_k")
                vT = bpool.tile([128, T], F16, tag="vT")
                for tc_i in range(NTC):
                    t0 = tc_i * TCH
                    xts = []
                    for c in range(8):
                        xt = xpool.tile([128, TCH], F16, tag="xt")
                        nc.sync.dma_start(
                            out=xt[:, :],
                            in_=x_t[b, c * 128:(c + 1) * 128, t0:t0 + TCH])
                        xts.append(xt)
                    if b == 0:
                        nc.sync.dma_start(out=cq_sb[:, t0:t0 + TCH],
                                          in_=cq_c[:, t0:t0 + TCH])
                        nc.sync.dma_start(out=sq_sb[:, t0:t0 + TCH],
                                          in_=sq_c[:, t0:t0 + TCH])
                        nc.sync.dma_start(out=ck_sb[:, t0:t0 + TCH],
                                          in_=ck_c[:, t0:t0 + TCH])
                        nc.sync.dma_start(out=sk_sb[:, t0:t0 + TCH],
                                          in_=sk_c[:, t0:t0 + TCH])
                    ps = []
                    for m in range(3):  # q, k, v
                        p = mmpool.tile([128, TCH], F32, tag="mm")
                        for c in range(8):
                            nc.tensor.matmul(
                                p[:, :],
                                w_sb[c][:, m * 128:(m + 1) * 128],
                                xts[c][:, :],
                                start=(c == 0), stop=(c == 7))
                        ps.append(p)
                    # q/k to SBUF, pair-swapped copies via strided DMA
                    q_sb = tpool.tile([128, TCH], F16, tag="qsb")
                    nc.vector.tensor_copy(q_sb[:, :], ps[0][:, :])
                    k_sb = tpool.tile([128, TCH], F16, tag="ksb")
                    nc.vector.tensor_copy(k_sb[:, :], ps[1][:, :])
                    qs_sb = tpool.tile([128, TCH], F16, tag="qssb")
                    nc.sync.dma_start(out=qs_sb[0::2, :], in_=q_sb[1::2, :])
                    nc.sync.dma_start(out=qs_sb[1::2, :], in_=q_sb[0::2, :])
                    ks_sb = tpool.tile([128, TCH], F16, tag="kssb")
                    nc.sync.dma_start(out=ks_sb[0::2, :], in_=k_sb[1::2, :])
                    nc.sync.dma_start(out=ks_sb[1::2, :], in_=k_sb[0::2, :])
                    # rope_q = q*cq + qs*sq ; rope_k = k*ck + ks*sk
                    tq1 = tpool.tile([128, TCH], F16, tag="tmp")
                    nc.vector.tensor_mul(tq1[:, :], q_sb[:, :],
                                         cq_sb[:, t0:t0 + TCH])
                    tq2 = tpool.tile([128, TCH], F16, tag="tmp")
                    nc.vector.tensor_mul(tq2[:, :], qs_sb[:, :],
                                         sq_sb[:, t0:t0 + TCH])
                    nc.vector.tensor_add(rope_q[:, t0:t0 + TCH], tq1[:, :],
                                         tq2[:, :])
                    tk1 = tpool.tile([128, TCH], F16, tag="tmp")
                    nc.vector.tensor_mul(tk1[:, :], k_sb[:, :],
                                         ck_sb[:, t0:t0 + TCH])
                    tk2 = tpool.tile([128, TCH], F16, tag="tmp")
                    nc.vector.tensor_mul(tk2[:, :], ks_sb[:, :],
                                         sk_sb[:, t0:t0 + TCH])
                    nc.vector.tensor_add(rope_k[:, t0:t0 + TCH], tk1[:, :],
                                         tk2[:, :])
                    nc.vector.tensor_copy(vT[:, t0:t0 + TCH], ps[2][:, :])

                # ---- V transpose into [tk, d]+ones layout ------------------
                vaug = []
                for h in range(HPC):
                    va = bpool.tile([128, NBLK * 64], F16, tag=f"vaug{h}")
                    for Tt in range(NBLK):
                        tp = tppool.tile([128, 64], F16, tag="tp")
                        nc.tensor.transpose(
                            tp[:, :],
                            vT[h * 64:(h + 1) * 64, Tt * 128:(Tt + 1) * 128],
                            ident_sb[h * 64:(h + 1) * 64, 0:64])
                        nc.scalar.copy(va[:, Tt * 64:Tt * 64 + 64], tp[:, :])
                    vaug.append(va)

                if debug and b == 0:
                    nc.sync.dma_start(out=dbg["rope_q"][:, :], in_=rope_q[:, :])
                    nc.sync.dma_start(out=dbg["rope_k"][:, :], in_=rope_k[:, :])
                    nc.sync.dma_start(out=dbg["vaug"][:, :], in_=vaug[0][:, :])

                # ---- attention + output projection per query group ---------
                for g in range(NTC):
                    q0 = g * TCH
                    y_pair = ypool_sb.tile([128, TCH], F16, tag="ypair")
                    for h in range(HPC):
                        y_ps = ypool.tile([64, TCH], F32, tag="y")
                        den_ps = dpool.tile([64, TCH], F32, tag="d")
                        ntk = 4 * g + 4
                        for Tt in range(ntk):
                            s_ps = mmpool.tile([128, TCH], F32, tag="mm")
                            diag = (Tt // 4 == g)
                            r = Tt % 4
                            c0 = 128 * r if diag else 0
                            nc.tensor.matmul(
                                s_ps[:, c0:TCH],
                                rope_k[h * 64:(h + 1) * 64,
                                       Tt * 128:(Tt + 1) * 128],
                                rope_q[h * 64:(h + 1) * 64,
                                       q0 + c0:q0 + TCH],
                                start=True, stop=True)
                            e_sb = epool.tile([128, TCH], F16, tag="exp")
                            if diag and r > 0:
                                nc.vector.memset(e_sb[:, 0:c0], 0.0)
                            nc.scalar.activation(
                                e_sb[:, c0:TCH], s_ps[:, c0:TCH],
                                mybir.ActivationFunctionType.Exp)
                            if diag:
                                nc.vector.tensor_mul(
                                    e_sb[:, c0:c0 + 128], e_sb[:, c0:c0 + 128],
                                    mask_sb[0][:, 0:128])
                            if debug and b == 0 and g == 0 and h == 0 and Tt == 0:
                                nc.sync.dma_start(out=dbg["exp"][:, :],
                                                  in_=e_sb[:, :])
                            nc.tensor.matmul(
                                y_ps[:, :],
                                vaug[h][:, Tt * 64:(Tt + 1) * 64],
                                e_sb[:, :],
                                start=(Tt == 0), stop=(Tt == ntk - 1))
                            nc.tensor.matmul(
                                den_ps[:, :], ones_sb[:, :], e_sb[:, :],
                                start=(Tt == 0), stop=(Tt == ntk - 1))
                        dbc = ypool_sb.tile([64, TCH], F32, tag="dbc")
                        nc.scalar.copy(dbc[:, :], den_ps[:, :])
                        rbc = ypool_sb.tile([64, TCH], F32, tag="rbc")
                        rsc = ypool_sb.tile([64, TCH], F32, tag="rsc")
                        nc.vector.reciprocal_approx_accurate(
                            rbc[:, :], dbc[:, :], rsc[:, :])
                        nc.vector.scalar_tensor_tensor(
                            y_pair[h * 64:(h + 1) * 64, :], y_ps[0:64, :],
                            1.0, rbc[:, :],
                            op0=mybir.AluOpType.mult, op1=mybir.AluOpType.mult)
                        if debug and b == 0 and g == 0 and h == 0:
                            nc.sync.dma_start(out=dbg["den"][:, :],
                                              in_=dbc[:, :])
                            nc.sync.dma_start(out=dbg["rbc"][:, :],
                                              in_=rbc[:, :])
                            nc.sync.dma_start(out=dbg["yh"][:, :],
                                              in_=y_pair[0:64, :])

                    j = b * NTC + g
                    last = False
                    for o in range(8):
                        op_ps = mmpool.tile([128, TCH], F32, tag="mm")
                        nc.tensor.matmul(
                            op_ps[:, :],
                            wp_sb[:, o * 128:(o + 1) * 128],
                            y_pair[:, :],
                            start=True, stop=True)
                        op_sb = tpool.tile([128, TCH], F16, tag="osb")
                        if o % 2 == 0:
                            nc.vector.tensor_copy(op_sb[:, :], op_ps[:, :])
                        else:
                            nc.scalar.copy(op_sb[:, :], op_ps[:, :])
                        if not last:
                            nc.sync.dma_start(
                                out=cc_in[j][o * 128:(o + 1) * 128, :],
                                in_=op_sb[:, :])
                        else:
                            for hh in range(2):
                                nc.sync.dma_start(
                                    out=cc_in_half[hh][o * 128:(o + 1) * 128, :],
                                    in_=op_sb[:, hh * 256:(hh + 1) * 256])
                    if not last:
                        nc.gpsimd.collective_compute(
                            "ReduceScatter", mybir.AluOpType.add,
                            replica_groups=groups,
                            ins=[cc_in[j].ap().opt()],
                            outs=[cc_out[j].ap().opt()])
                        nc.sync.dma_start(out=out_ext[j, :, :],
                                          in_=cc_out[j][:, :])
                    else:
                        for hh in range(2):
                            c0, c1_ = hh * 256, (hh + 1) * 256
                            nc.gpsimd.collective_compute(
                                "ReduceScatter", mybir.AluOpType.add,
                                replica_groups=groups,
                                ins=[cc_in_half[hh].ap().opt()],
                                outs=[cc_half[hh].ap().opt()])
                            nc.sync.dma_start(out=out_ext[j, :, c0:c1_],
                                              in_=cc_half[hh][:, :])
    if not nc.is_finalized():
        nc.finalize()
    return nc


_NC_CACHE = None


def _get_nc():
    global _NC_CACHE
    if _NC_CACHE is None:
        _NC_CACHE = build()
    return _NC_CACHE


def make_in_maps(x, w_qkv, w_proj):
    x_t = np.ascontiguousarray(np.asarray(x, np.float32).transpose(0, 2, 1)).astype(np.float16)
    w_qkv = np.asarray(w_qkv, np.float32)
    w_proj = np.asarray(w_proj, np.float32)
    in_maps = []
    for r in range(NCORE):
        ha, hb = 2 * r, 2 * r + 1
        qrows = (list(range(ha * 64, ha * 64 + 64))
                 + list(range(hb * 64, hb * 64 + 64)))
        qsrows = ([ha * 64 + (d ^ 1) for d in range(64)]
                  + [hb * 64 + (d ^ 1) for d in range(64)])
        rows = (qrows + [1024 + i for i in qrows] + [2048 + i for i in qrows])
        w_all = np.ascontiguousarray(w_qkv[rows, :].T).astype(np.float16)
        w_p = np.ascontiguousarray(
            w_proj[:, ha * 64:(hb + 1) * 64].T).astype(np.float16)
        in_maps.append({"x_t": x_t, "w_all": w_all, "w_p": w_p})
    return in_maps


def assemble(results):
    outT = np.zeros((B, C, T), np.float32)
    for r in range(NCORE):
        o = results[r]["out"].astype(np.float32)
        for b in range(B):
            for g in range(NTC):
                outT[b, r * 128:(r + 1) * 128, g * TCH:(g + 1) * TCH] = \
                    o[b * NTC + g]
    return np.ascontiguousarray(outT.transpose(0, 2, 1))


def run(x, w_qkv, w_proj, trace=False):
    nc = _get_nc()
    in_maps = make_in_maps(x, w_qkv, w_proj)
    res = run_bass_kernel_spmd(nc, in_maps, list(range(NCORE)), trace=trace)
    return assemble(res.results), res


def kernel(x, w_qkv, w_proj):
    out, _ = run(x, w_qkv, w_proj, trace=False)
    return out

